# revision 26
# baseline (speedup 1.0000x reference)
"""Enformer multi-head attention (relative position) Trainium2 Bass kernel.

Problem: nn_Enformer_20753281974759
  B=2, L=1536, D_MODEL=1536, H=8, DQK=64, DV=192, POS_FEATS=192

Sharding (8 NeuronCores): data-parallel over batch x tensor-parallel over
heads.  Core c handles batch b = c // 4 and heads (2*(c%4), 2*(c%4)+1).
Each core computes a full-width [L, D_MODEL] partial of the output
projection (its 2 heads' contribution); the host sums the 4 partials per
batch and adds bo.

All matmuls run in bf16 (fp32 PSUM accumulation); softmax runs without
max-subtraction (logits are bounded ~|25| here, exp is safe in fp32) and
is split as exp(content+rel) = exp(content)*exp(rel):

  phase 1:  Q^T/K^T = W^T x^T (head-dim on partitions), V = x Wv
  phase 1b: rK^T = Wrk^T pe^T (pe is a host-computed constant)
  phase 2:  per (head, 128-query tile):
            rel-window matmul -> exp on PSUM-evict -> diagonal-shift DMA;
            content matmul -> exp on PSUM-evict; one DVE op forms
            w = expC * expRel AND the softmax denominator (accum_out);
            normalize in place; XBAR DMA-transpose straight into the
            [j-part, i-col] layout the attention matmul needs (no PE
            transposes, no PSUM round-trip); attn^T accumulated over the
            12 j-blocks, evicted into ATT.
  phase 3:  out_partial = attn Wo_c from the ATT tiles kept in SBUF
"""

import math
from contextlib import ExitStack

import numpy as np

import concourse.bacc as bacc
import concourse.mybir as mybir
import concourse.tile as tile
import bass_rust
from concourse import bass_utils

# ---------------------------------------------------------------- constants
B, L, DM = 2, 1536, 1536
H, DQK, DV = 8, 64, 192
PF = 192                  # POS_FEATS
R = 2 * L - 1             # 3071 relative positions
P = 128
KO = DM // P              # 12 contraction chunks
NQ = L // P               # 12 query tiles
HL = 2                    # heads per core
DH = HL * DQK             # 128: stacked head dim on partitions
DVL = HL * DV             # 384: local value dim
N_CORES = 8
SCALE = DQK ** -0.5
WREL = L + P - 1          # 1663: rel window width per query tile
WRELP = WREL + 1          # padded to even
NEG_INF = -3.0e38

_F32 = mybir.dt.float32
_BF16 = mybir.dt.bfloat16

# row permutation of the per-core Wo slice matching ATT's on-chip layout:
# ATT row (chunk*128 + p) holds global-v ATT_PERM[chunk*128 + p]
ATT_PERM = np.concatenate(
    [
        np.arange(0, 128),        # chunk0: head0 v 0..127
        np.arange(192, 320),      # chunk1: head1 v 0..127
        np.arange(320, 384),      # chunk2[0:64]: head1 v 128..191
        np.arange(128, 192),      # chunk2[64:128]: head0 v 128..191
    ]
)


# ------------------------------------------------------------ pos features
def _pos_features_np():
    """Enformer positional basis [2L-1, PF], matched to the jax-on-CPU
    reference (same float32 op sequence on the CPU backend); numpy
    fallback if jax is unavailable."""
    try:
        import jax
        import jax.numpy as jnp

        cpu = jax.devices("cpu")[0]
        with jax.default_device(cpu):
            pos = jnp.arange(-L + 1, L, dtype=jnp.float32)
            ap = jnp.abs(pos)[:, None]
            nb = PF // 6
            Lf = float(L)
            max_range = np.log(Lf) / np.log(2.0)
            half_life = jnp.asarray(
                2.0 ** np.linspace(3.0, max_range, nb), jnp.float32
            )
            f_exp = jnp.exp(-np.log(2.0) / half_life * ap)
            cw = jnp.asarray(2.0 ** np.arange(1, nb + 1) - 1.0, jnp.float32)
            f_cm = (cw[None, :] > ap).astype(jnp.float32)
            stddev = Lf / (2 * nb)
            mean = jnp.asarray(np.linspace(Lf / nb, Lf, nb), jnp.float32)
            conc = (mean / stddev) ** 2
            rate = mean / stddev**2
            log_unnorm = jnp.where(
                ap > 0, (conc - 1.0) * jnp.log(jnp.maximum(ap, 1e-20)), 0.0
            ) - rate * ap
            log_norm = jax.lax.lgamma(conc) - conc * jnp.log(rate)
            p = jnp.exp(log_unnorm - log_norm) + 1e-8
            f_g = p / jnp.max(p)
            emb = jnp.concatenate([f_exp, f_cm, f_g], axis=-1)
            out = jnp.concatenate([emb, jnp.sign(pos)[:, None] * emb], axis=-1)
            return np.asarray(out, dtype=np.float32)
    except Exception:
        ft = np.float32
        pos = np.arange(-L + 1, L, dtype=ft)
        ap = np.abs(pos)[:, None]
        nb = PF // 6
        Lf = float(L)
        max_range = np.log(Lf) / np.log(2.0)
        half_life = (2.0 ** np.linspace(3.0, max_range, nb)).astype(ft)
        f_exp = np.exp((-ft(np.log(2.0)) / half_life) * ap)
        cw = (2.0 ** np.arange(1, nb + 1) - 1.0).astype(ft)
        f_cm = (cw[None, :] > ap).astype(ft)
        stddev = Lf / (2 * nb)
        mean = np.linspace(Lf / nb, Lf, nb)
        conc = (np.asarray(mean, ft) / ft(stddev)) ** 2
        rate = np.asarray(mean, ft) / ft(stddev) ** 2
        lg = np.array([math.lgamma(float(c)) for c in conc], dtype=ft)
        log_unnorm = np.where(
            ap > 0, (conc - ft(1.0)) * np.log(np.maximum(ap, ft(1e-20))), ft(0.0)
        ) - rate * ap
        log_norm = lg - conc * np.log(rate)
        p = np.exp(log_unnorm - log_norm) + ft(1e-8)
        f_g = p / p.max()
        emb = np.concatenate([f_exp, f_cm, f_g], axis=-1)
        return np.concatenate(
            [emb, np.sign(pos)[:, None] * emb], axis=-1
        ).astype(np.float32)


# ------------------------------------------------------------- bass builder
def _diag_ap(ap, width, out_cols, shift0):
    """AP reading src[p, shift0 - p + j] (j in [0, out_cols)) from a 2-D
    [P, width] SBUF tile: flat element step width-1 walks one partition
    down while stepping one element back."""
    c = ap.copy()
    c.ap = bass_rust.VecI64Pair([[width - 1, P], [1, out_cols]])
    c.offset = ap.offset + shift0
    return c


def build_kernel(loop_T=None):
    nc = bacc.Bacc("TRN2", target_bir_lowering=False, debug=False,
                   num_devices=N_CORES)

    xT = nc.dram_tensor("xT", [DM, L], _BF16, kind="ExternalInput")
    wq = nc.dram_tensor("wq", [DM, DH], _BF16, kind="ExternalInput")
    wk = nc.dram_tensor("wk", [DM, DH], _BF16, kind="ExternalInput")
    wv = nc.dram_tensor("wv", [DM, DVL], _BF16, kind="ExternalInput")
    wrk = nc.dram_tensor("wrk", [PF, DH], _BF16, kind="ExternalInput")
    peT = nc.dram_tensor("peT", [PF, R], _BF16, kind="ExternalInput")
    bw = nc.dram_tensor("bw", [DH, 1], _F32, kind="ExternalInput")
    br = nc.dram_tensor("br", [DH, 1], _F32, kind="ExternalInput")
    wo = nc.dram_tensor("wo", [DVL, DM], _BF16, kind="ExternalInput")
    y = nc.dram_tensor("y", [L, DM], _BF16, kind="ExternalOutput")

    with tile.TileContext(nc) as tc, ExitStack() as ctx:
        persist = ctx.enter_context(tc.tile_pool(name="persist", bufs=1))

        QwT = persist.tile([P, L], _BF16)       # Q^T*scale + r_w_bias
        QrT = persist.tile([P, L], _BF16)       # Q^T*scale + r_r_bias
        KT = persist.tile([P, L], _BF16)
        rKT = persist.tile([P, R + 1], _BF16)
        Vg = persist.tile([P, KO, DVL], _BF16)  # V rows j, cols h0 v0:192|h1 v0:192
        ATT = persist.tile([P, 3, L], _BF16)    # attn^T (normalized)
        bwT = persist.tile([P, 1], _F32)
        brT = persist.tile([P, 1], _F32)
        wo_sb = persist.tile([P, 3, DM], _BF16)

        nc.sync.dma_start(bwT[:], bw[:])
        nc.sync.dma_start(brT[:], br[:])

        import contextlib
        loop_cm = tc.For_i(0, loop_T, 1) if loop_T else contextlib.nullcontext()
        ctx.enter_context(loop_cm)

        # xT and wv stay alive into phase 2 (V projection is interleaved
        # there), so they live in a kernel-scope pool.
        xpool = ctx.enter_context(tc.tile_pool(name="xTp", bufs=1))
        xT_sb = xpool.tile([P, KO, L], _BF16)
        wv_sb = xpool.tile([P, KO, DVL], _BF16)

        # ---------------- phase 1: rK + Q/K projections ----------------
        with ExitStack() as p1:
            wpool = p1.enter_context(tc.tile_pool(name="wproj", bufs=1))
            pepool = p1.enter_context(tc.tile_pool(name="pe", bufs=1))
            qk_ps = p1.enter_context(
                tc.tile_pool(name="qk_ps", bufs=1, space="PSUM")
            )
            rps = p1.enter_context(tc.tile_pool(name="r_ps", bufs=2, space="PSUM"))

            # DMA order matters: small weight tensors first so the early
            # matmuls are not stuck behind the 4.7 MB xT stream.
            wq_sb = wpool.tile([P, KO, DH], _BF16)
            wk_sb = wpool.tile([P, KO, DH], _BF16)
            pe0 = pepool.tile([P, R + 1], _BF16)
            pe1 = pepool.tile([PF - P, R + 1], _BF16)
            wrk0 = pepool.tile([P, DH], _BF16)
            wrk1 = pepool.tile([PF - P, DH], _BF16)
            nc.sync.dma_start(wrk0[:], wrk[:P, :])
            nc.sync.dma_start(wrk1[:], wrk[P:, :])
            for rc in range(6):
                lo = rc * 512
                w = min(512, R - lo)
                nc.sync.dma_start(pe0[:, lo : lo + w], peT[:P, lo : lo + w])
                nc.sync.dma_start(pe1[:, lo : lo + w], peT[P:, lo : lo + w])
            nc.sync.dma_start(wq_sb[:], wq[:].rearrange("(ko p) d -> p ko d", p=P))
            nc.sync.dma_start(wk_sb[:], wk[:].rearrange("(ko p) d -> p ko d", p=P))
            xTr = xT[:].rearrange("(ko p) i -> p ko i", p=P)
            for mo in range(KO):
                nc.sync.dma_start(xT_sb[:, mo, :], xTr[:, mo, :])
            nc.scalar.dma_start(wv_sb[:], wv[:].rearrange("(ko p) d -> p ko d", p=P))
            nc.scalar.dma_start(wo_sb[:], wo[:].rearrange("(c p) n -> p c n", p=P))

            zcol = pepool.tile([P, 1], _BF16)
            nc.vector.memset(zcol[:], 0.0)
            nc.vector.tensor_copy(pe0[:, R : R + 1], zcol[:])
            nc.vector.tensor_copy(pe1[:, R : R + 1], zcol[: PF - P])

            # rKT = Wrk^T @ peT (first: its inputs arrive before xT)
            for rc in range(6):
                lo = rc * 512
                ps = rps.tile([P, 512], _F32)
                nc.tensor.matmul(
                    ps[:], wrk0[:], pe0[:, lo : lo + 512], start=True, stop=False
                )
                nc.tensor.matmul(
                    ps[:], wrk1[:], pe1[:, lo : lo + 512], start=False, stop=True
                )
                nc.scalar.activation(
                    rKT[:, lo : lo + 512], ps[:],
                    mybir.ActivationFunctionType.Identity,
                )

            # Q^T and K^T: [DH=128, L], accumulated over 12 m-chunks that
            # chase the xT DMA stream
            qt_ps = [qk_ps.tile([P, 512], _F32, tag=f"qt{c}", name=f"qt{c}") for c in range(3)]
            kt_ps = [qk_ps.tile([P, 512], _F32, tag=f"kt{c}", name=f"kt{c}") for c in range(3)]
            for mo in range(KO):
                for c in range(3):
                    sl = slice(c * 512, (c + 1) * 512)
                    nc.tensor.matmul(
                        qt_ps[c][:], wq_sb[:, mo, :], xT_sb[:, mo, sl],
                        start=(mo == 0), stop=(mo == KO - 1),
                    )
                    nc.tensor.matmul(
                        kt_ps[c][:], wk_sb[:, mo, :], xT_sb[:, mo, sl],
                        start=(mo == 0), stop=(mo == KO - 1),
                    )
            for c in range(3):
                sl = slice(c * 512, (c + 1) * 512)
                nc.scalar.activation(
                    QwT[:, sl], qt_ps[c][:],
                    mybir.ActivationFunctionType.Identity,
                    bias=bwT[:], scale=SCALE,
                )
                nc.scalar.activation(
                    QrT[:, sl], qt_ps[c][:],
                    mybir.ActivationFunctionType.Identity,
                    bias=brT[:], scale=SCALE,
                )
                nc.vector.tensor_copy(KT[:, sl], kt_ps[c][:])
        # ---------------- phase 2+3: attention + output projection ----------
        # Per (head, group-of-4 query tiles):
        #   per query tile iq: rel-window matmuls -> exp-evict (ACT) ->
        #   diagonal-shift DMA; content matmuls -> exp-evict (ACT);
        #   w = expC*expRel + row-sum in one DVE op; 1/sum; in-place
        #   normalize; XBAR DMA-transpose into wT4 [j-part, 12 blocks,
        #   512 i].  attn^T = V-chunk.T @ wT4 accumulated over the 12
        #   j-blocks, evicted into ATT.
        # Software pipelining: attn matmuls for a (h, g) issue one h-slot
        # later (during the next chain block) so the PE never stalls on the
        # exp->mult->normalize->transpose chain; the output projection for
        # group g issues piecewise once both heads' ATT columns landed.
        with ExitStack() as p2:
            relwin_pool = p2.enter_context(tc.tile_pool(name="relwin", bufs=4))
            relsh_pool = p2.enter_context(tc.tile_pool(name="relsh", bufs=4))
            ec_pool = p2.enter_context(tc.tile_pool(name="expc", bufs=4))
            s_pool = p2.enter_context(tc.tile_pool(name="sS", bufs=4))
            wt4_pool = p2.enter_context(tc.tile_pool(name="wt4", bufs=4))
            small_pool = p2.enter_context(tc.tile_pool(name="small", bufs=8))
            obuf = p2.enter_context(tc.tile_pool(name="obuf", bufs=4))
            # shared scratch PSUM: rel-window chunks, V accumulation, and
            # the interleaved output projection all draw from 3 banks
            sc_ps = p2.enter_context(tc.tile_pool(name="sc_ps", bufs=3, space="PSUM"))
            s_ps = p2.enter_context(tc.tile_pool(name="s_ps", bufs=1, space="PSUM"))
            a_ps = p2.enter_context(tc.tile_pool(name="a_ps", bufs=1, space="PSUM"))
            rel_ps = sc_ps

            def v_block(jo):
                # one 128-row tile of the V projection (interleaved into the
                # first head-slot's chains to fill the pipeline warm-up)
                v_full = sc_ps.tile([P, 512], _F32, tag="sc", name="v_full")
                v_ps = v_full[:, :DVL]
                for mo in range(KO):
                    nc.tensor.matmul(
                        v_ps[:],
                        xT_sb[:, mo, jo * P : (jo + 1) * P],
                        wv_sb[:, mo, :],
                        start=(mo == 0), stop=(mo == KO - 1),
                    )
                nc.vector.tensor_copy(Vg[:, jo, :], v_ps[:])

            def iq_chain(g, h, it, wT4):
                hsl = slice(h * DQK, (h + 1) * DQK)
                iq = g * 4 + it
                i0 = iq * P
                isl = slice(i0, i0 + P)
                c0 = (L - P) - i0  # rel-window start in rel coords

                # rel window -> exp -> [P, WRELP] bf16
                relwinE = relwin_pool.tile([P, WRELP], _BF16)
                relshE = relsh_pool.tile([P, L], _BF16)
                for rc in range(4):
                    lo = rc * 512
                    w = min(512, WRELP - lo)
                    ps = rel_ps.tile([P, 512], _F32, tag="sc", name="relps")
                    nc.tensor.matmul(
                        ps[:, :w], QrT[hsl, isl],
                        rKT[hsl, c0 + lo : c0 + lo + w],
                        start=True, stop=True,
                    )
                    nc.scalar.activation(
                        relwinE[:, lo : lo + w], ps[:, :w],
                        mybir.ActivationFunctionType.Exp,
                    )

                # diagonal shift: relshE[p, j] = relwinE[p, 127-p+j]
                nc.sync.dma_start(
                    relshE[:], _diag_ap(relwinE[:], WRELP, L, P - 1)
                )

                # content -> exp (one ACT op over the 3-bank PSUM tile)
                expC = ec_pool.tile([P, L], _BF16)
                sps = s_ps.tile([P, 3 * 512], _F32, tag="s", name="sps")
                for jc in range(3):
                    sl = slice(jc * 512, (jc + 1) * 512)
                    nc.tensor.matmul(
                        sps[:, sl], QwT[hsl, isl], KT[hsl, sl],
                        start=True, stop=True,
                    )
                nc.scalar.activation(
                    expC[:], sps[:], mybir.ActivationFunctionType.Exp,
                )

                # w = expC * expRel; denominator via accum_out
                wt = s_pool.tile([P, L], _BF16)
                su = small_pool.tile([P, 1], _F32, tag="su")
                nc.vector.scalar_tensor_tensor(
                    wt[:], expC[:], 1.0, relshE[:],
                    mybir.AluOpType.mult, mybir.AluOpType.mult,
                    accum_out=su[:],
                )
                rec = small_pool.tile([P, 1], _F32, tag="rec")
                nc.vector.reciprocal(rec[:], su[:])
                nc.vector.tensor_scalar_mul(wt[:], wt[:], rec[:])

                # XBAR transpose: wT4[p, jb, it*128+c] = wt[c, jb*128+p]
                nc.sync.dma_start(
                    wT4[:, :, it * P : (it + 1) * P], wt[:], transpose=True,
                )

            def issue_attn_part(h, g, wT4, jb_lo, jb_hi, psA, psB):
                # a slice of the attn^T accumulation (jb_lo..jb_hi-1); the
                # 24 matmuls are spread through the next group's chains so
                # the in-order PE never parks on a not-yet-transposed wT4
                if h == 0:
                    psBv = psB[:]
                    lhsB = lambda jb: Vg[:, jb, 64:192]
                else:
                    psBv = psB[:64, :]
                    lhsB = lambda jb: Vg[:, jb, 320:384]
                for jb in range(jb_lo, jb_hi):
                    nc.tensor.matmul(
                        psA[:], Vg[:, jb, h * DV : h * DV + 128],
                        wT4[:, jb, :],
                        start=(jb == 0), stop=(jb == NQ - 1),
                    )
                    nc.tensor.matmul(
                        psBv, lhsB(jb), wT4[:, jb, :],
                        start=(jb == 0), stop=(jb == NQ - 1),
                    )

            def issue_attn_evict(h, g, psA, psB):
                # ATT row layout (= ATT_PERM on the host):
                #   chunk0       = h0 v[0:128)   <- psA(h0)
                #   chunk1       = h1 v[0:128)   <- psA(h1)
                #   chunk2[64:]  = h0 v[128:192) <- psB(h0)[64:128]
                #   chunk2[0:64] = h1 v[128:192) <- psB(h1)[0:64]
                gsl = slice(g * 512, (g + 1) * 512)
                nc.vector.tensor_copy(ATT[:, h, gsl], psA[:])
                if h == 0:
                    nc.vector.tensor_copy(ATT[64:128, 2, gsl], psB[64:128, :])
                else:
                    nc.vector.tensor_copy(ATT[0:64, 2, gsl], psB[0:64, :])

            def issue_p3(iq):
                isl = slice(iq * P, (iq + 1) * P)
                ob = obuf.tile([P, DM], _BF16)
                for nck in range(3):
                    nsl = slice(nck * 512, (nck + 1) * 512)
                    p3t = sc_ps.tile([P, 512], _F32, tag="sc", name="p3t")
                    for c in range(3):
                        nc.tensor.matmul(
                            p3t[:], ATT[:, c, isl], wo_sb[:, c, nsl],
                            start=(c == 0), stop=(c == 2),
                        )
                    nc.vector.tensor_copy(ob[:, nsl], p3t[:])
                nc.sync.dma_start(y[isl, :], ob[:])

            p3_queue = []
            vq = list(range(KO))
            prev = None  # (g-1, [wT4_h0, wT4_h1])

            def attn_step(it, prev_g, prev_w):
                # prev h0 spread over it0-1, prev h1 over it2-3, with the
                # ATT evictions at the half/end boundaries
                ph = it // 2
                half = it % 2
                if half == 0:
                    psA = a_ps.tile([P, 4 * P], _F32, tag="A", name="psA")
                    psB = a_ps.tile([P, 4 * P], _F32, tag="B", name="psB")
                    attn_step.ps = (psA, psB)
                psA, psB = attn_step.ps
                issue_attn_part(ph, prev_g, prev_w[ph],
                                6 * half, 6 * half + 6, psA, psB)
                if half == 1:
                    issue_attn_evict(ph, prev_g, psA, psB)
                    if ph == 1:
                        p3_queue.extend(range(prev_g * 4, prev_g * 4 + 4))

            for g in range(3):
                wT4h = [
                    wt4_pool.tile([P, NQ, 4 * P], _BF16, tag="wt4",
                                  name=f"wT4_{g}_{h}")
                    for h in range(HL)
                ]
                for it in range(4):
                    for h in range(HL):
                        iq_chain(g, h, it, wT4h[h])
                        if g == 0 and vq:
                            # fill the pipeline warm-up with the V projection
                            for _ in range(2 if len(vq) > 4 else 1):
                                if vq:
                                    v_block(vq.pop(0))
                    if p3_queue:
                        issue_p3(p3_queue.pop(0))
                    if prev is not None:
                        attn_step(it, prev[0], prev[1])
                prev = (g, wT4h)
            # drain: attn for the last group, then its output projection
            for it in range(4):
                if p3_queue:
                    issue_p3(p3_queue.pop(0))
                attn_step(it, prev[0], prev[1])
            for iq in p3_queue:
                issue_p3(iq)

    nc.compile()
    return nc


# ------------------------------------------------------------------ runner
_CACHE = {}


def _get_nc():
    if "nc" not in _CACHE:
        _CACHE["nc"] = build_kernel()
        _CACHE["peT"] = np.ascontiguousarray(_pos_features_np().T)
    return _CACHE["nc"], _CACHE["peT"]


def _bf16(a):
    import ml_dtypes

    return np.ascontiguousarray(a).astype(ml_dtypes.bfloat16)


def make_in_maps(x, Wq, Wk, Wv, Wrk, r_w_bias, r_r_bias, Wo, peT):
    peT_b = _bf16(peT)
    in_maps = []
    for c in range(N_CORES):
        b, hp = divmod(c, 4)
        h0 = 2 * hp
        qsl = slice(h0 * DQK, h0 * DQK + DH)
        vsl = slice(h0 * DV, h0 * DV + DVL)
        in_maps.append(
            {
                "xT": _bf16(x[b].T),
                "wq": _bf16(Wq[:, qsl]),
                "wk": _bf16(Wk[:, qsl]),
                "wv": _bf16(Wv[:, vsl]),
                "wrk": _bf16(Wrk[:, qsl]),
                "peT": peT_b,
                "bw": np.ascontiguousarray(
                    r_w_bias[0, h0 : h0 + HL, 0, :].reshape(DH, 1)
                ).astype(np.float32),
                "br": np.ascontiguousarray(
                    r_r_bias[0, h0 : h0 + HL, 0, :].reshape(DH, 1)
                ).astype(np.float32),
                "wo": _bf16(Wo[vsl, :][ATT_PERM, :]),
            }
        )
    return in_maps


def kernel(x, Wq, Wk, Wv, Wrk, r_w_bias, r_r_bias, Wo, bo, **run_kwargs):
    x = np.asarray(x, np.float32)
    Wq = np.asarray(Wq, np.float32)
    Wk = np.asarray(Wk, np.float32)
    Wv = np.asarray(Wv, np.float32)
    Wrk = np.asarray(Wrk, np.float32)
    r_w_bias = np.asarray(r_w_bias, np.float32)
    r_r_bias = np.asarray(r_r_bias, np.float32)
    Wo = np.asarray(Wo, np.float32)
    bo = np.asarray(bo, np.float32)

    nc, peT = _get_nc()
    in_maps = make_in_maps(x, Wq, Wk, Wv, Wrk, r_w_bias, r_r_bias, Wo, peT)
    res = bass_utils.run_bass_kernel_spmd(
        nc, in_maps, core_ids=list(range(N_CORES)), **run_kwargs
    )
    out = np.zeros((B, L, DM), np.float32)
    for c in range(N_CORES):
        out[c // 4] += np.asarray(res.results[c]["y"], np.float32)
    out += bo[None, None, :]
    if run_kwargs:
        _CACHE["last_results"] = res
    return out


# revision 27
# speedup vs baseline: 1.2545x; 1.2545x over previous
"""Enformer multi-head attention (relative position) Trainium2 Bass kernel.

Problem: nn_Enformer_20753281974759
  B=2, L=1536, D_MODEL=1536, H=8, DQK=64, DV=192, POS_FEATS=192

Sharding (8 NeuronCores): data-parallel over batch x tensor-parallel over
heads.  Core c handles batch b = c // 4 and heads (2*(c%4), 2*(c%4)+1).
Each core computes a full-width [L, D_MODEL] partial of the output
projection (its 2 heads' contribution); the host sums the 4 partials per
batch and adds bo.

All matmuls run in bf16 (fp32 PSUM accumulation); softmax runs without
max-subtraction (logits are bounded ~|25| here, exp is safe in fp32) and
is split as exp(content+rel) = exp(content)*exp(rel):

  phase 1:  Q^T/K^T = W^T x^T (head-dim on partitions), V = x Wv
  phase 1b: rK^T = Wrk^T pe^T (pe is a host-computed constant)
  phase 2:  per (head, 128-query tile):
            rel-window matmul -> exp on PSUM-evict -> diagonal-shift DMA;
            content matmul -> exp on PSUM-evict; one DVE op forms
            w = expC * expRel AND the softmax denominator (accum_out);
            normalize in place; XBAR DMA-transpose straight into the
            [j-part, i-col] layout the attention matmul needs (no PE
            transposes, no PSUM round-trip); attn^T accumulated over the
            12 j-blocks, evicted into ATT.
  phase 3:  out_partial = attn Wo_c from the ATT tiles kept in SBUF
"""

import math
from contextlib import ExitStack

import numpy as np

import concourse.bacc as bacc
import concourse.mybir as mybir
import concourse.tile as tile
import bass_rust
from concourse import bass_utils

# ---------------------------------------------------------------- constants
B, L, DM = 2, 1536, 1536
H, DQK, DV = 8, 64, 192
PF = 192                  # POS_FEATS
R = 2 * L - 1             # 3071 relative positions
P = 128
KO = DM // P              # 12 contraction chunks
NQ = L // P               # 12 query tiles
HL = 2                    # heads per core
DH = HL * DQK             # 128: stacked head dim on partitions
DVL = HL * DV             # 384: local value dim
N_CORES = 8
SCALE = DQK ** -0.5
WREL = L + P - 1          # 1663: rel window width per query tile
WRELP = WREL + 1          # padded to even
NEG_INF = -3.0e38

_F32 = mybir.dt.float32
_BF16 = mybir.dt.bfloat16

# row permutation of the per-core Wo slice matching ATT's on-chip layout:
# ATT row (chunk*128 + p) holds global-v ATT_PERM[chunk*128 + p]
ATT_PERM = np.concatenate(
    [
        np.arange(0, 128),        # chunk0: head0 v 0..127
        np.arange(192, 320),      # chunk1: head1 v 0..127
        np.arange(320, 384),      # chunk2[0:64]: head1 v 128..191
        np.arange(128, 192),      # chunk2[64:128]: head0 v 128..191
    ]
)


# ------------------------------------------------------------ pos features
def _pos_features_np():
    """Enformer positional basis [2L-1, PF], matched to the jax-on-CPU
    reference (same float32 op sequence on the CPU backend); numpy
    fallback if jax is unavailable."""
    try:
        import jax
        import jax.numpy as jnp

        cpu = jax.devices("cpu")[0]
        with jax.default_device(cpu):
            pos = jnp.arange(-L + 1, L, dtype=jnp.float32)
            ap = jnp.abs(pos)[:, None]
            nb = PF // 6
            Lf = float(L)
            max_range = np.log(Lf) / np.log(2.0)
            half_life = jnp.asarray(
                2.0 ** np.linspace(3.0, max_range, nb), jnp.float32
            )
            f_exp = jnp.exp(-np.log(2.0) / half_life * ap)
            cw = jnp.asarray(2.0 ** np.arange(1, nb + 1) - 1.0, jnp.float32)
            f_cm = (cw[None, :] > ap).astype(jnp.float32)
            stddev = Lf / (2 * nb)
            mean = jnp.asarray(np.linspace(Lf / nb, Lf, nb), jnp.float32)
            conc = (mean / stddev) ** 2
            rate = mean / stddev**2
            log_unnorm = jnp.where(
                ap > 0, (conc - 1.0) * jnp.log(jnp.maximum(ap, 1e-20)), 0.0
            ) - rate * ap
            log_norm = jax.lax.lgamma(conc) - conc * jnp.log(rate)
            p = jnp.exp(log_unnorm - log_norm) + 1e-8
            f_g = p / jnp.max(p)
            emb = jnp.concatenate([f_exp, f_cm, f_g], axis=-1)
            out = jnp.concatenate([emb, jnp.sign(pos)[:, None] * emb], axis=-1)
            return np.asarray(out, dtype=np.float32)
    except Exception:
        ft = np.float32
        pos = np.arange(-L + 1, L, dtype=ft)
        ap = np.abs(pos)[:, None]
        nb = PF // 6
        Lf = float(L)
        max_range = np.log(Lf) / np.log(2.0)
        half_life = (2.0 ** np.linspace(3.0, max_range, nb)).astype(ft)
        f_exp = np.exp((-ft(np.log(2.0)) / half_life) * ap)
        cw = (2.0 ** np.arange(1, nb + 1) - 1.0).astype(ft)
        f_cm = (cw[None, :] > ap).astype(ft)
        stddev = Lf / (2 * nb)
        mean = np.linspace(Lf / nb, Lf, nb)
        conc = (np.asarray(mean, ft) / ft(stddev)) ** 2
        rate = np.asarray(mean, ft) / ft(stddev) ** 2
        lg = np.array([math.lgamma(float(c)) for c in conc], dtype=ft)
        log_unnorm = np.where(
            ap > 0, (conc - ft(1.0)) * np.log(np.maximum(ap, ft(1e-20))), ft(0.0)
        ) - rate * ap
        log_norm = lg - conc * np.log(rate)
        p = np.exp(log_unnorm - log_norm) + ft(1e-8)
        f_g = p / p.max()
        emb = np.concatenate([f_exp, f_cm, f_g], axis=-1)
        return np.concatenate(
            [emb, np.sign(pos)[:, None] * emb], axis=-1
        ).astype(np.float32)


# ------------------------------------------------------------- bass builder
def _diag_ap(ap, width, out_cols, shift0):
    """AP reading src[p, shift0 - p + j] (j in [0, out_cols)) from a 2-D
    [P, width] SBUF tile: flat element step width-1 walks one partition
    down while stepping one element back."""
    c = ap.copy()
    c.ap = bass_rust.VecI64Pair([[width - 1, P], [1, out_cols]])
    c.offset = ap.offset + shift0
    return c


def build_kernel(loop_T=None):
    nc = bacc.Bacc("TRN2", target_bir_lowering=False, debug=False,
                   num_devices=N_CORES)

    xT = nc.dram_tensor("xT", [DM, L], _BF16, kind="ExternalInput")
    wq = nc.dram_tensor("wq", [DM, DH], _BF16, kind="ExternalInput")
    wk = nc.dram_tensor("wk", [DM, DH], _BF16, kind="ExternalInput")
    wv = nc.dram_tensor("wv", [DM, DVL], _BF16, kind="ExternalInput")
    wrk = nc.dram_tensor("wrk", [PF, DH], _BF16, kind="ExternalInput")
    peT = nc.dram_tensor("peT", [PF, R], _BF16, kind="ExternalInput")
    bw = nc.dram_tensor("bw", [DH, 1], _F32, kind="ExternalInput")
    br = nc.dram_tensor("br", [DH, 1], _F32, kind="ExternalInput")
    wo = nc.dram_tensor("wo", [DVL, DM], _BF16, kind="ExternalInput")
    y = nc.dram_tensor("y", [L, DM], _BF16, kind="ExternalOutput")

    with tile.TileContext(nc) as tc, ExitStack() as ctx:
        persist = ctx.enter_context(tc.tile_pool(name="persist", bufs=1))

        QwT = persist.tile([P, L], _BF16)       # Q^T*scale + r_w_bias
        QrT = persist.tile([P, L], _BF16)       # Q^T*scale + r_r_bias
        KT = persist.tile([P, L], _BF16)
        rKT = persist.tile([P, R + 1], _BF16)
        Vg = persist.tile([P, KO, DVL], _BF16)  # V rows j, cols h0 v0:192|h1 v0:192
        ATT = persist.tile([P, 3, L], _BF16)    # attn^T (normalized)
        bwT = persist.tile([P, 1], _F32)
        brT = persist.tile([P, 1], _F32)
        wo_sb = persist.tile([P, 3, DM], _BF16)

        nc.sync.dma_start(bwT[:], bw[:])
        nc.sync.dma_start(brT[:], br[:])

        import contextlib
        loop_cm = tc.For_i(0, loop_T, 1) if loop_T else contextlib.nullcontext()
        ctx.enter_context(loop_cm)

        # xT and wv stay alive into phase 2 (V projection is interleaved
        # there), so they live in a kernel-scope pool.
        xpool = ctx.enter_context(tc.tile_pool(name="xTp", bufs=1))
        xT_sb = xpool.tile([P, KO, L], _BF16)
        wv_sb = xpool.tile([P, KO, DVL], _BF16)

        # ---------------- phase 1: rK + Q/K projections ----------------
        with ExitStack() as p1:
            wpool = p1.enter_context(tc.tile_pool(name="wproj", bufs=1))
            pepool = p1.enter_context(tc.tile_pool(name="pe", bufs=1))
            qk_ps = p1.enter_context(
                tc.tile_pool(name="qk_ps", bufs=1, space="PSUM")
            )
            rps = p1.enter_context(tc.tile_pool(name="r_ps", bufs=2, space="PSUM"))

            # DMA order matters: small weight tensors first so the early
            # matmuls are not stuck behind the 4.7 MB xT stream.
            wq_sb = wpool.tile([P, KO, DH], _BF16)
            wk_sb = wpool.tile([P, KO, DH], _BF16)
            pe0 = pepool.tile([P, R + 1], _BF16)
            pe1 = pepool.tile([PF - P, R + 1], _BF16)
            wrk0 = pepool.tile([P, DH], _BF16)
            wrk1 = pepool.tile([PF - P, DH], _BF16)
            nc.sync.dma_start(wrk0[:], wrk[:P, :])
            nc.sync.dma_start(wrk1[:], wrk[P:, :])
            for rc in range(6):
                lo = rc * 512
                w = min(512, R - lo)
                nc.sync.dma_start(pe0[:, lo : lo + w], peT[:P, lo : lo + w])
                nc.sync.dma_start(pe1[:, lo : lo + w], peT[P:, lo : lo + w])
            nc.sync.dma_start(wq_sb[:], wq[:].rearrange("(ko p) d -> p ko d", p=P))
            nc.sync.dma_start(wk_sb[:], wk[:].rearrange("(ko p) d -> p ko d", p=P))
            xTr = xT[:].rearrange("(ko p) i -> p ko i", p=P)
            for mo in range(KO):
                nc.sync.dma_start(xT_sb[:, mo, :], xTr[:, mo, :])
            nc.scalar.dma_start(wv_sb[:], wv[:].rearrange("(ko p) d -> p ko d", p=P))
            nc.scalar.dma_start(wo_sb[:], wo[:].rearrange("(c p) n -> p c n", p=P))

            zcol = pepool.tile([P, 1], _BF16)
            nc.vector.memset(zcol[:], 0.0)
            nc.vector.tensor_copy(pe0[:, R : R + 1], zcol[:])
            nc.vector.tensor_copy(pe1[:, R : R + 1], zcol[: PF - P])

            # rKT = Wrk^T @ peT (first: its inputs arrive before xT)
            for rc in range(6):
                lo = rc * 512
                ps = rps.tile([P, 512], _F32)
                nc.tensor.matmul(
                    ps[:], wrk0[:], pe0[:, lo : lo + 512], start=True, stop=False
                )
                nc.tensor.matmul(
                    ps[:], wrk1[:], pe1[:, lo : lo + 512], start=False, stop=True
                )
                nc.scalar.activation(
                    rKT[:, lo : lo + 512], ps[:],
                    mybir.ActivationFunctionType.Identity,
                )

            # Q^T and K^T: [DH=128, L], accumulated over 12 m-chunks that
            # chase the xT DMA stream
            qt_ps = [qk_ps.tile([P, 512], _F32, tag=f"qt{c}", name=f"qt{c}") for c in range(3)]
            kt_ps = [qk_ps.tile([P, 512], _F32, tag=f"kt{c}", name=f"kt{c}") for c in range(3)]
            for mo in range(KO):
                for c in range(3):
                    sl = slice(c * 512, (c + 1) * 512)
                    nc.tensor.matmul(
                        qt_ps[c][:], wq_sb[:, mo, :], xT_sb[:, mo, sl],
                        start=(mo == 0), stop=(mo == KO - 1),
                    )
                    nc.tensor.matmul(
                        kt_ps[c][:], wk_sb[:, mo, :], xT_sb[:, mo, sl],
                        start=(mo == 0), stop=(mo == KO - 1),
                    )
            for c in range(3):
                sl = slice(c * 512, (c + 1) * 512)
                nc.scalar.activation(
                    QwT[:, sl], qt_ps[c][:],
                    mybir.ActivationFunctionType.Identity,
                    bias=bwT[:], scale=SCALE,
                )
                nc.scalar.activation(
                    QrT[:, sl], qt_ps[c][:],
                    mybir.ActivationFunctionType.Identity,
                    bias=brT[:], scale=SCALE,
                )
                nc.vector.tensor_copy(KT[:, sl], kt_ps[c][:])
        # ---------------- phase 2+3: attention + output projection ----------
        # Per (head, group-of-4 query tiles):
        #   per query tile iq: rel-window matmuls -> exp-evict (ACT) ->
        #   diagonal-shift DMA; content matmuls -> exp-evict (ACT);
        #   w = expC*expRel + row-sum in one DVE op; 1/sum; in-place
        #   normalize; XBAR DMA-transpose into wT4 [j-part, 12 blocks,
        #   512 i].  attn^T = V-chunk.T @ wT4 accumulated over the 12
        #   j-blocks, evicted into ATT.
        # Software pipelining: attn matmuls for a (h, g) issue one h-slot
        # later (during the next chain block) so the PE never stalls on the
        # exp->mult->normalize->transpose chain; the output projection for
        # group g issues piecewise once both heads' ATT columns landed.
        with ExitStack() as p2:
            relwin_pool = p2.enter_context(tc.tile_pool(name="relwin", bufs=4))
            relsh_pool = p2.enter_context(tc.tile_pool(name="relsh", bufs=4))
            ec_pool = p2.enter_context(tc.tile_pool(name="expc", bufs=4))
            s_pool = p2.enter_context(tc.tile_pool(name="sS", bufs=4))
            wt4_pool = p2.enter_context(tc.tile_pool(name="wt4", bufs=4))
            small_pool = p2.enter_context(tc.tile_pool(name="small", bufs=8))
            obuf = p2.enter_context(tc.tile_pool(name="obuf", bufs=4))
            # shared scratch PSUM: rel-window chunks, V accumulation, and
            # the interleaved output projection all draw from 3 banks
            sc_ps = p2.enter_context(tc.tile_pool(name="sc_ps", bufs=3, space="PSUM"))
            s_ps = p2.enter_context(tc.tile_pool(name="s_ps", bufs=3, space="PSUM"))
            a_ps = p2.enter_context(tc.tile_pool(name="a_ps", bufs=1, space="PSUM"))
            rel_ps = sc_ps

            def v_block(jo):
                # one 128-row tile of the V projection (interleaved into the
                # first head-slot's chains to fill the pipeline warm-up)
                v_full = sc_ps.tile([P, 512], _F32, tag="sc", name="v_full")
                v_ps = v_full[:, :DVL]
                for mo in range(KO):
                    nc.tensor.matmul(
                        v_ps[:],
                        xT_sb[:, mo, jo * P : (jo + 1) * P],
                        wv_sb[:, mo, :],
                        start=(mo == 0), stop=(mo == KO - 1),
                    )
                nc.vector.tensor_copy(Vg[:, jo, :], v_ps[:])

            def iq_chain(g, h, it, wT4):
                hsl = slice(h * DQK, (h + 1) * DQK)
                iq = g * 4 + it
                i0 = iq * P
                isl = slice(i0, i0 + P)
                c0 = (L - P) - i0  # rel-window start in rel coords

                # rel window -> exp -> [P, WRELP] bf16
                relwinE = relwin_pool.tile([P, WRELP], _BF16)
                relshE = relsh_pool.tile([P, L], _BF16)
                for rc in range(4):
                    lo = rc * 512
                    w = min(512, WRELP - lo)
                    ps = rel_ps.tile([P, 512], _F32, tag="sc", name="relps")
                    nc.tensor.matmul(
                        ps[:, :w], QrT[hsl, isl],
                        rKT[hsl, c0 + lo : c0 + lo + w],
                        start=True, stop=True,
                    )
                    nc.scalar.activation(
                        relwinE[:, lo : lo + w], ps[:, :w],
                        mybir.ActivationFunctionType.Exp,
                    )

                # diagonal shift: relshE[p, j] = relwinE[p, 127-p+j]
                nc.sync.dma_start(
                    relshE[:], _diag_ap(relwinE[:], WRELP, L, P - 1)
                )

                # content -> exp (chunked so the two interleaved heads
                # share the 3 content PSUM banks)
                expC = ec_pool.tile([P, L], _BF16)
                for jc in range(3):
                    sl = slice(jc * 512, (jc + 1) * 512)
                    sps = s_ps.tile([P, 512], _F32, tag="s", name="sps")
                    nc.tensor.matmul(
                        sps[:], QwT[hsl, isl], KT[hsl, sl],
                        start=True, stop=True,
                    )
                    nc.scalar.activation(
                        expC[:, sl], sps[:], mybir.ActivationFunctionType.Exp,
                    )

                # w = expC * expRel; denominator via accum_out
                wt = s_pool.tile([P, L], _BF16)
                su = small_pool.tile([P, 1], _F32, tag="su")
                nc.vector.scalar_tensor_tensor(
                    wt[:], expC[:], 1.0, relshE[:],
                    mybir.AluOpType.mult, mybir.AluOpType.mult,
                    accum_out=su[:],
                )
                rec = small_pool.tile([P, 1], _F32, tag="rec")
                nc.vector.reciprocal(rec[:], su[:])
                nc.vector.tensor_scalar_mul(wt[:], wt[:], rec[:])

                # XBAR transpose: wT4[p, jb, it*128+c] = wt[c, jb*128+p]
                nc.sync.dma_start(
                    wT4[:, :, it * P : (it + 1) * P], wt[:], transpose=True,
                )

            def issue_attn_part(h, g, wT4, jb_lo, jb_hi, psA, psB):
                # a slice of the attn^T accumulation (jb_lo..jb_hi-1); the
                # 24 matmuls are spread through the next group's chains so
                # the in-order PE never parks on a not-yet-transposed wT4
                if h == 0:
                    psBv = psB[:]
                    lhsB = lambda jb: Vg[:, jb, 64:192]
                else:
                    psBv = psB[:64, :]
                    lhsB = lambda jb: Vg[:, jb, 320:384]
                for jb in range(jb_lo, jb_hi):
                    nc.tensor.matmul(
                        psA[:], Vg[:, jb, h * DV : h * DV + 128],
                        wT4[:, jb, :],
                        start=(jb == 0), stop=(jb == NQ - 1),
                    )
                    nc.tensor.matmul(
                        psBv, lhsB(jb), wT4[:, jb, :],
                        start=(jb == 0), stop=(jb == NQ - 1),
                    )

            def issue_attn_evict(h, g, psA, psB):
                # ATT row layout (= ATT_PERM on the host):
                #   chunk0       = h0 v[0:128)   <- psA(h0)
                #   chunk1       = h1 v[0:128)   <- psA(h1)
                #   chunk2[64:]  = h0 v[128:192) <- psB(h0)[64:128]
                #   chunk2[0:64] = h1 v[128:192) <- psB(h1)[0:64]
                gsl = slice(g * 512, (g + 1) * 512)
                nc.scalar.activation(
                    ATT[:, h, gsl], psA[:],
                    mybir.ActivationFunctionType.Identity,
                )
                if h == 0:
                    nc.vector.tensor_copy(ATT[64:128, 2, gsl], psB[64:128, :])
                else:
                    nc.vector.tensor_copy(ATT[0:64, 2, gsl], psB[0:64, :])

            def issue_p3(iq):
                isl = slice(iq * P, (iq + 1) * P)
                ob = obuf.tile([P, DM], _BF16)
                for nck in range(3):
                    nsl = slice(nck * 512, (nck + 1) * 512)
                    p3t = sc_ps.tile([P, 512], _F32, tag="sc", name="p3t")
                    for c in range(3):
                        nc.tensor.matmul(
                            p3t[:], ATT[:, c, isl], wo_sb[:, c, nsl],
                            start=(c == 0), stop=(c == 2),
                        )
                    if nck % 2 == 0:
                        nc.vector.tensor_copy(ob[:, nsl], p3t[:])
                    else:
                        nc.scalar.activation(
                            ob[:, nsl], p3t[:],
                            mybir.ActivationFunctionType.Identity,
                        )
                nc.sync.dma_start(y[isl, :], ob[:])

            p3_queue = []
            vq = list(range(KO))
            prev = None  # (g-1, [wT4_h0, wT4_h1])

            def attn_step(it, prev_g, prev_w):
                # prev h0 spread over it0-1, prev h1 over it2-3, with the
                # ATT evictions at the half/end boundaries
                ph = it // 2
                half = it % 2
                if half == 0:
                    psA = a_ps.tile([P, 4 * P], _F32, tag="A", name="psA")
                    psB = a_ps.tile([P, 4 * P], _F32, tag="B", name="psB")
                    attn_step.ps = (psA, psB)
                psA, psB = attn_step.ps
                issue_attn_part(ph, prev_g, prev_w[ph],
                                6 * half, 6 * half + 6, psA, psB)
                if half == 1:
                    issue_attn_evict(ph, prev_g, psA, psB)
                    if ph == 1:
                        p3_queue.extend(range(prev_g * 4, prev_g * 4 + 4))

            for g in range(3):
                wT4h = [
                    wt4_pool.tile([P, NQ, 4 * P], _BF16, tag="wt4",
                                  name=f"wT4_{g}_{h}")
                    for h in range(HL)
                ]
                for it in range(4):
                    for h in range(HL):
                        iq_chain(g, h, it, wT4h[h])
                        if g == 0 and vq:
                            # fill the pipeline warm-up with the V projection
                            for _ in range(2 if len(vq) > 4 else 1):
                                if vq:
                                    v_block(vq.pop(0))
                    if p3_queue:
                        issue_p3(p3_queue.pop(0))
                    if prev is not None:
                        attn_step(it, prev[0], prev[1])
                prev = (g, wT4h)
            # drain: attn for the last group, then its output projection
            for it in range(4):
                if p3_queue:
                    issue_p3(p3_queue.pop(0))
                attn_step(it, prev[0], prev[1])
            for iq in p3_queue:
                issue_p3(iq)

    nc.compile()
    return nc


# ------------------------------------------------------------------ runner
_CACHE = {}


def _get_nc():
    if "nc" not in _CACHE:
        _CACHE["nc"] = build_kernel()
        _CACHE["peT"] = np.ascontiguousarray(_pos_features_np().T)
    return _CACHE["nc"], _CACHE["peT"]


def _bf16(a):
    import ml_dtypes

    return np.ascontiguousarray(a).astype(ml_dtypes.bfloat16)


def make_in_maps(x, Wq, Wk, Wv, Wrk, r_w_bias, r_r_bias, Wo, peT):
    peT_b = _bf16(peT)
    in_maps = []
    for c in range(N_CORES):
        b, hp = divmod(c, 4)
        h0 = 2 * hp
        qsl = slice(h0 * DQK, h0 * DQK + DH)
        vsl = slice(h0 * DV, h0 * DV + DVL)
        in_maps.append(
            {
                "xT": _bf16(x[b].T),
                "wq": _bf16(Wq[:, qsl]),
                "wk": _bf16(Wk[:, qsl]),
                "wv": _bf16(Wv[:, vsl]),
                "wrk": _bf16(Wrk[:, qsl]),
                "peT": peT_b,
                "bw": np.ascontiguousarray(
                    r_w_bias[0, h0 : h0 + HL, 0, :].reshape(DH, 1)
                ).astype(np.float32),
                "br": np.ascontiguousarray(
                    r_r_bias[0, h0 : h0 + HL, 0, :].reshape(DH, 1)
                ).astype(np.float32),
                "wo": _bf16(Wo[vsl, :][ATT_PERM, :]),
            }
        )
    return in_maps


def kernel(x, Wq, Wk, Wv, Wrk, r_w_bias, r_r_bias, Wo, bo, **run_kwargs):
    x = np.asarray(x, np.float32)
    Wq = np.asarray(Wq, np.float32)
    Wk = np.asarray(Wk, np.float32)
    Wv = np.asarray(Wv, np.float32)
    Wrk = np.asarray(Wrk, np.float32)
    r_w_bias = np.asarray(r_w_bias, np.float32)
    r_r_bias = np.asarray(r_r_bias, np.float32)
    Wo = np.asarray(Wo, np.float32)
    bo = np.asarray(bo, np.float32)

    nc, peT = _get_nc()
    in_maps = make_in_maps(x, Wq, Wk, Wv, Wrk, r_w_bias, r_r_bias, Wo, peT)
    res = bass_utils.run_bass_kernel_spmd(
        nc, in_maps, core_ids=list(range(N_CORES)), **run_kwargs
    )
    out = np.zeros((B, L, DM), np.float32)
    for c in range(N_CORES):
        out[c // 4] += np.asarray(res.results[c]["y"], np.float32)
    out += bo[None, None, :]
    if run_kwargs:
        _CACHE["last_results"] = res
    return out
